# revision 13
# baseline (speedup 1.0000x reference)
"""BatchAllTripletLoss on 8 Trainium2 NeuronCores.

Strategy
-------
The loss  sum_{i,j,k} relu(d(i,j) - d(i,k) + m) * mask / (count + eps)  is
invariant to batch permutation, so the host sorts the batch by label; every
class becomes one contiguous column slice.  Core c owns the 64 sorted anchors
[64c, 64c+64).  All mask logic (class membership, j!=i diagonal) is carried
by per-core int8 mask tensors, so one compiled SPMD program serves all cores.

Per core, on device:
  1. column norms via Square + ones-matmul (bf16 inputs, f32 accumulate)
  2. G = Xanch @ X^T (bf16 PE matmul), D = 1 - G * invn_i * invn_j
  3. POS[i,q] = D[i, class_slice(i)] compacted by per-class predicated
     copies; NEG[i,k] = D[i,k] - margin with same-class columns -> +1e9
     (margin folded into NEG so POS bias needs no add)
  4. main loop over stacked bias columns (each anchor appears twice, on
     partitions p and p+64, taking even/odd positives -> all 128 lanes):
     ScalarE: relu(bias - NEG) with free-dim accumulation
     VectorE: count(NEG < bias) with free-dim accumulation
  5. per-core [sum, count] partials via ones-matmul; host sums and divides

The B^3 triplet tensor is never materialized; the main loop touches
64*88*512 = 2.9M elements per core per pass.
"""

import numpy as np

B, D, NCORES = 512, 768, 8
MA = 64  # anchors per core
MARGIN = 0.5
EPS = 1e-8
BIG = 1e9

_PROG_CACHE: dict = {}


class Plan:
    pass


def _make_plan(labels: np.ndarray) -> Plan:
    p = Plan()
    order = np.argsort(labels, kind="stable")
    lab = labels[order]
    nclass = int(lab.max()) + 1
    counts = np.bincount(lab, minlength=nclass).astype(int)
    n = [int(c) for c in counts if c > 0]
    starts = np.concatenate([[0], np.cumsum(n)]).astype(int)
    cls_of = np.searchsorted(starts, np.arange(B), side="right") - 1

    Kpos = max(n)
    Kpos2 = Kpos + (Kpos % 2)
    J2 = Kpos2 // 2

    posmask = np.zeros((NCORES, MA, Kpos2), dtype=np.int8)
    negmask = np.zeros((NCORES, MA, B), dtype=np.int8)
    pm7 = np.zeros((NCORES, len(n), MA, Kpos2), dtype=np.int8)
    for c in range(NCORES):
        for r in range(MA):
            a = MA * c + r
            i = cls_of[a]
            s, nk = starts[i], n[i]
            posmask[c, r, :nk] = 1
            posmask[c, r, a - s] = 0  # j == i
            negmask[c, r, :] = 1
            negmask[c, r, s : s + nk] = 0
            pm7[c, i, r, :] = posmask[c, r, :]

    p.order = order
    p.n = n
    p.starts = starts
    p.Kpos2 = Kpos2
    p.J2 = J2
    p.posmask = posmask
    p.negmask = negmask
    p.pm7 = pm7
    p.key = tuple(n)
    return p


def _build_program(p: Plan):
    from contextlib import ExitStack

    import concourse.bacc as bacc
    import concourse.mybir as mybir
    import concourse.tile as tile

    f32 = mybir.dt.float32
    bf16 = mybir.dt.bfloat16
    i8 = mybir.dt.int8
    Alu = mybir.AluOpType
    Act = mybir.ActivationFunctionType
    X = mybir.AxisListType.X

    J2, Kpos2 = p.J2, p.Kpos2
    NCLS = len(p.n)
    NCH = D // 128

    nc = bacc.Bacc("TRN2", target_bir_lowering=False, debug=False, num_devices=NCORES)

    xT = nc.dram_tensor("xT", [D, B], bf16, kind="ExternalInput").ap()
    xaT = nc.dram_tensor("xaT", [D, MA], bf16, kind="ExternalInput").ap()
    xa = nc.dram_tensor("xa", [MA, D], bf16, kind="ExternalInput").ap()
    pm7 = nc.dram_tensor("pm7", [NCLS, MA, Kpos2], i8, kind="ExternalInput").ap()
    nm = nc.dram_tensor("nm", [MA, B], i8, kind="ExternalInput").ap()
    out = nc.dram_tensor("out", [1, 2], f32, kind="ExternalOutput").ap()

    with tile.TileContext(nc) as tc, ExitStack() as ctx:
        pool = ctx.enter_context(tc.tile_pool(name="sb", bufs=1))
        sqpool = ctx.enter_context(tc.tile_pool(name="sq", bufs=3))
        scrA = ctx.enter_context(tc.tile_pool(name="scrA", bufs=4))
        scrV = ctx.enter_context(tc.tile_pool(name="scrV", bufs=4))
        pp = ctx.enter_context(tc.tile_pool(name="ps", bufs=1, space="PSUM"))

        ones_bf = pool.tile([128, 1], bf16)
        nc.gpsimd.memset(ones_bf[:], 1.0)
        ones_f32 = pool.tile([128, 1], f32)
        nc.gpsimd.memset(ones_f32[:], 1.0)
        ones_row = pool.tile([1, MA], f32)
        nc.gpsimd.memset(ones_row[:], 1.0)

        # ---- loads (per-chunk so squares/matmuls pipeline) --------------
        xTv = xT.rearrange("(c p) j -> p c j", p=128)
        xT_t = pool.tile([128, NCH, B], bf16)
        for q in range(NCH):
            nc.sync.dma_start(xT_t[:, q, :], xTv[:, q, :])
        xaTv = xaT.rearrange("(c p) j -> p c j", p=128)
        xaT_t = pool.tile([128, NCH, MA], bf16)
        nc.sync.dma_start(xaT_t[:], xaTv)
        xa_t = pool.tile([MA, D], bf16)
        nc.sync.dma_start(xa_t[:], xa)
        pm7_t = pool.tile([MA, NCLS, Kpos2], i8)
        nc.sync.dma_start(pm7_t[:], pm7.rearrange("k m q -> m k q"))
        nm_t = pool.tile([MA, B], i8)
        nc.sync.dma_start(nm_t[:], nm)

        # ---- column norms ssq[j] = sum_d x[d,j]^2 -----------------------
        ps_ssq = pp.tile([1, B], f32)
        for q in range(NCH):
            sq = sqpool.tile([128, B], bf16, tag="sq")
            nc.scalar.activation(sq[:], xT_t[:, q, :], Act.Square)
            nc.tensor.matmul(
                ps_ssq[:], ones_bf[:], sq[:], start=(q == 0), stop=(q == NCH - 1)
            )
        nrm = pool.tile([1, B], f32)
        nc.scalar.activation(nrm[:], ps_ssq[:], Act.Sqrt)
        invn = pool.tile([1, B], f32)
        nc.vector.reciprocal(invn[:], nrm[:])

        # ---- anchor norms ----------------------------------------------
        scr_a = pool.tile([MA, D], bf16)
        ssqa = pool.tile([MA, 1], f32)
        nc.scalar.activation(scr_a[:], xa_t[:], Act.Square, accum_out=ssqa[:])
        nrma = pool.tile([MA, 1], f32)
        nc.scalar.activation(nrma[:], ssqa[:], Act.Sqrt)
        invna = pool.tile([MA, 1], f32)
        nc.vector.reciprocal(invna[:], nrma[:])

        # ---- S = G*invna*invn (the "1 -" of cosine distance cancels in
        # d_ij - d_ik, so we work with similarities directly:
        # t = d_ij - d_ik + m = (m - S_ij) + S_ik) ------------------------
        ps_G = pp.tile([MA, B], f32)
        for q in range(NCH):
            nc.tensor.matmul(
                ps_G[:], xaT_t[:, q, :], xT_t[:, q, :],
                start=(q == 0), stop=(q == NCH - 1),
            )
        ps_B = pp.tile([MA, B], f32)
        nc.tensor.matmul(ps_B[:], ones_row[:], invn[:], start=True, stop=True)
        invnB = pool.tile([MA, B], f32)
        nc.scalar.activation(invnB[:], ps_B[:], Act.Copy)
        Sm = pool.tile([MA, B], bf16)
        nc.vector.scalar_tensor_tensor(
            Sm[:], ps_G[:], invna[:], invnB[:], Alu.mult, Alu.mult
        )
        ms = pool.tile([MA, B], f32)
        nc.vector.tensor_scalar(ms[:], Sm[:], -1.0, MARGIN, Alu.mult, Alu.add)

        # ---- POS bias = m - S_ij (compacted, data-driven classes) -------
        posf = pool.tile([MA, Kpos2], f32)
        nc.gpsimd.memset(posf[:], -BIG)
        for i in range(NCLS):
            s, nk = p.starts[i], p.n[i]
            nc.vector.copy_predicated(
                posf[:, 0:nk], pm7_t[:, i, 0:nk], ms[:, s : s + nk]
            )
        POSst = pool.tile([128, J2], f32)
        nc.gpsimd.memset(POSst[:], -BIG)
        pe = posf.rearrange("p (a two) -> p two a", two=2)
        nc.vector.tensor_copy(POSst[0:MA, :], pe[:, 0, :])
        nc.sync.dma_start(POSst[64 : 64 + MA, :], pe[:, 1, :])

        # ---- NEG = S_ik (dense bf16; same-class columns -> -BIG) --------
        NEGS = pool.tile([128, B], bf16)
        nc.gpsimd.memset(NEGS[:], -BIG)
        nc.vector.copy_predicated(NEGS[0:MA, :], nm_t[:], Sm[:])
        nc.sync.dma_start(NEGS[64 : 64 + MA, :], NEGS[0:MA, :])

        # negated bias for the count pass: t>0  <=>  NEGS > -bias
        POSng = pool.tile([128, J2], f32)
        nc.vector.tensor_scalar_mul(POSng[:], POSst[:], -1.0)

        # ---- main loop ---------------------------------------------------
        # count: self-accumulating on DVE (one scalar_tensor_tensor per jj,
        #   acc += (NEGS > -bias); bf16 integers stay exact up to 256)
        # relu: ACT or DVE (split for balance), PE matmul-accumulates the
        #   bf16 relu tiles into one PSUM bank via a ones-vector contraction
        cnt_acc = pool.tile([128, B], bf16)
        nc.gpsimd.memset(cnt_acc[:], 0.0)
        ps_sum = pp.tile([1, B], f32)
        for jj in range(J2):
            if jj % 7 < 4:
                sA = scrA.tile([128, B], bf16, tag="sA")
                nc.scalar.activation(
                    sA[:], NEGS[:], Act.Relu, bias=POSst[:, jj : jj + 1]
                )
            else:
                sA = scrV.tile([128, B], bf16, tag="sV")
                nc.vector.tensor_scalar(
                    sA[:], NEGS[:], POSst[:, jj : jj + 1], 0.0, Alu.add, Alu.max
                )
            nc.tensor.matmul(
                ps_sum[:], ones_bf[:], sA[:],
                start=(jj == 0), stop=(jj == J2 - 1), skip_group_check=True,
            )
            nc.vector.scalar_tensor_tensor(
                cnt_acc[:], NEGS[:], POSng[:, jj : jj + 1], cnt_acc[:],
                Alu.is_gt, Alu.add,
            )

        # ---- final reduction --------------------------------------------
        ps_cnt = pp.tile([1, B], f32)
        nc.tensor.matmul(ps_cnt[:], ones_bf[:], cnt_acc[:], start=True, stop=True)
        outs = pool.tile([1, 2], f32)
        scr1 = pool.tile([1, B], f32)
        nc.scalar.activation(scr1[:], ps_sum[:], Act.Copy, accum_out=outs[:, 0:1])
        scr2 = pool.tile([1, B], f32)
        nc.scalar.activation(scr2[:], ps_cnt[:], Act.Copy, accum_out=outs[:, 1:2])
        nc.sync.dma_start(out, outs[:])

    nc.compile()
    return nc


def _in_maps(p: Plan, emb: np.ndarray):
    import ml_dtypes

    bf = ml_dtypes.bfloat16
    xs = np.ascontiguousarray(emb[p.order])
    xT = np.ascontiguousarray(xs.T.astype(bf))
    maps = []
    for c in range(NCORES):
        xa = xs[MA * c : MA * (c + 1)]
        maps.append(
            {
                "xT": xT,
                "xaT": np.ascontiguousarray(xa.T.astype(bf)),
                "xa": np.ascontiguousarray(xa.astype(bf)),
                "pm7": p.pm7[c],
                "nm": p.negmask[c],
            }
        )
    return maps


LAST_RESULT = None  # BassKernelResults of the most recent run (for profiling)


def kernel(embeddings, labels):
    global LAST_RESULT
    import os

    from concourse.bass_utils import run_bass_kernel_spmd

    emb = np.ascontiguousarray(np.asarray(embeddings, dtype=np.float32))
    lab = np.asarray(labels).astype(np.int64)
    p = _make_plan(lab)
    if p.key not in _PROG_CACHE:
        _PROG_CACHE[p.key] = _build_program(p)
    nc = _PROG_CACHE[p.key]
    trace = bool(int(os.environ.get("TRIPLET_TRACE", "0")))
    kw = {}
    if os.environ.get("TRIPLET_TMPDIR"):
        kw["tmpdir"] = os.environ["TRIPLET_TMPDIR"]
    LAST_RESULT = run_bass_kernel_spmd(
        nc, _in_maps(p, emb), list(range(NCORES)), trace=trace, **kw
    )
    res = LAST_RESULT.results
    S = 0.0
    C = 0.0
    for r in res:
        o = np.asarray(r["out"], dtype=np.float64).reshape(-1)
        S += o[0]
        C += o[1]
    return np.float32(S / (C + EPS))


# revision 15
# speedup vs baseline: 1.8767x; 1.8767x over previous
"""BatchAllTripletLoss on 8 Trainium2 NeuronCores.

Strategy
-------
The loss  sum_{i,j,k} relu(d(i,j) - d(i,k) + m) * mask / (count + eps)  is
invariant to batch permutation, so the host sorts the batch by label; every
class becomes one contiguous column slice.  Core c owns the 64 sorted anchors
[64c, 64c+64).  All mask logic (class membership, j!=i diagonal) is carried
by per-core int8 mask tensors, so one compiled SPMD program serves all cores.

Per core, on device:
  1. column norms via Square + ones-matmul (bf16 inputs, f32 accumulate)
  2. G = Xanch @ X^T (bf16 PE matmul), D = 1 - G * invn_i * invn_j
  3. POS[i,q] = D[i, class_slice(i)] compacted by per-class predicated
     copies; NEG[i,k] = D[i,k] - margin with same-class columns -> +1e9
     (margin folded into NEG so POS bias needs no add)
  4. main loop over stacked bias columns (each anchor appears twice, on
     partitions p and p+64, taking even/odd positives -> all 128 lanes):
     ScalarE: relu(bias - NEG) with free-dim accumulation
     VectorE: count(NEG < bias) with free-dim accumulation
  5. per-core [sum, count] partials via ones-matmul; host sums and divides

The B^3 triplet tensor is never materialized; the main loop touches
64*88*512 = 2.9M elements per core per pass.
"""

import numpy as np

B, D, NCORES = 512, 768, 8
MA = 64  # anchors per core
MARGIN = 0.5
EPS = 1e-8
BIG = 1e9

_PROG_CACHE: dict = {}


class Plan:
    pass


def _make_plan(labels: np.ndarray) -> Plan:
    p = Plan()
    order = np.argsort(labels, kind="stable")
    lab = labels[order]
    nclass = int(lab.max()) + 1
    counts = np.bincount(lab, minlength=nclass).astype(int)
    n = [int(c) for c in counts if c > 0]
    starts = np.concatenate([[0], np.cumsum(n)]).astype(int)
    cls_of = np.searchsorted(starts, np.arange(B), side="right") - 1

    Kpos = max(n)
    Kpos2 = Kpos + (Kpos % 2)
    J2 = Kpos2 // 2

    posmask = np.zeros((NCORES, MA, Kpos2), dtype=np.int8)
    negmask = np.zeros((NCORES, MA, B), dtype=np.int8)
    pm7 = np.zeros((NCORES, len(n), MA, Kpos2), dtype=np.int8)
    for c in range(NCORES):
        for r in range(MA):
            a = MA * c + r
            i = cls_of[a]
            s, nk = starts[i], n[i]
            posmask[c, r, :nk] = 1
            posmask[c, r, a - s] = 0  # j == i
            negmask[c, r, :] = 1
            negmask[c, r, s : s + nk] = 0
            pm7[c, i, r, :] = posmask[c, r, :]

    p.order = order
    p.n = n
    p.starts = starts
    p.Kpos2 = Kpos2
    p.J2 = J2
    p.posmask = posmask
    p.negmask = negmask
    p.pm7 = pm7
    # fast-path tables: full-width positive mask + per-anchor counts
    pm_full = np.zeros((NCORES, MA, B), dtype=np.int8)
    cnts = np.zeros((NCORES, MA, 4), dtype=np.float32)
    for c in range(NCORES):
        for r in range(MA):
            a = MA * c + r
            i = cls_of[a]
            s, nk = starts[i], n[i]
            pm_full[c, r, s : s + nk] = 1
            pm_full[c, r, a] = 0
            npos, nneg = nk - 1, B - nk
            cnts[c, r] = (npos, nneg, npos * nneg, MARGIN * npos)
    p.pm_full = pm_full
    p.cnts = cnts
    p.n_valid = int(cnts[:, :, 2].sum())
    p.key = tuple(n)
    return p


def _build_program_scan(p: Plan):
    from contextlib import ExitStack

    import concourse.bacc as bacc
    import concourse.mybir as mybir
    import concourse.tile as tile

    f32 = mybir.dt.float32
    bf16 = mybir.dt.bfloat16
    i8 = mybir.dt.int8
    Alu = mybir.AluOpType
    Act = mybir.ActivationFunctionType
    X = mybir.AxisListType.X

    J2, Kpos2 = p.J2, p.Kpos2
    NCLS = len(p.n)
    NCH = D // 128

    nc = bacc.Bacc("TRN2", target_bir_lowering=False, debug=False, num_devices=NCORES)

    xT = nc.dram_tensor("xT", [D, B], bf16, kind="ExternalInput").ap()
    xaT = nc.dram_tensor("xaT", [D, MA], bf16, kind="ExternalInput").ap()
    xa = nc.dram_tensor("xa", [MA, D], bf16, kind="ExternalInput").ap()
    pm7 = nc.dram_tensor("pm7", [NCLS, MA, Kpos2], i8, kind="ExternalInput").ap()
    nm = nc.dram_tensor("nm", [MA, B], i8, kind="ExternalInput").ap()
    out = nc.dram_tensor("out", [1, 2], f32, kind="ExternalOutput").ap()

    with tile.TileContext(nc) as tc, ExitStack() as ctx:
        pool = ctx.enter_context(tc.tile_pool(name="sb", bufs=1))
        sqpool = ctx.enter_context(tc.tile_pool(name="sq", bufs=3))
        scrA = ctx.enter_context(tc.tile_pool(name="scrA", bufs=4))
        scrV = ctx.enter_context(tc.tile_pool(name="scrV", bufs=4))
        pp = ctx.enter_context(tc.tile_pool(name="ps", bufs=1, space="PSUM"))

        ones_bf = pool.tile([128, 1], bf16)
        nc.gpsimd.memset(ones_bf[:], 1.0)
        ones_f32 = pool.tile([128, 1], f32)
        nc.gpsimd.memset(ones_f32[:], 1.0)
        ones_row = pool.tile([1, MA], f32)
        nc.gpsimd.memset(ones_row[:], 1.0)

        # ---- loads (per-chunk so squares/matmuls pipeline) --------------
        xTv = xT.rearrange("(c p) j -> p c j", p=128)
        xT_t = pool.tile([128, NCH, B], bf16)
        for q in range(NCH):
            nc.sync.dma_start(xT_t[:, q, :], xTv[:, q, :])
        xaTv = xaT.rearrange("(c p) j -> p c j", p=128)
        xaT_t = pool.tile([128, NCH, MA], bf16)
        nc.sync.dma_start(xaT_t[:], xaTv)
        xa_t = pool.tile([MA, D], bf16)
        nc.sync.dma_start(xa_t[:], xa)
        pm7_t = pool.tile([MA, NCLS, Kpos2], i8)
        nc.sync.dma_start(pm7_t[:], pm7.rearrange("k m q -> m k q"))
        nm_t = pool.tile([MA, B], i8)
        nc.sync.dma_start(nm_t[:], nm)

        # ---- column norms ssq[j] = sum_d x[d,j]^2 -----------------------
        ps_ssq = pp.tile([1, B], f32)
        for q in range(NCH):
            sq = sqpool.tile([128, B], bf16, tag="sq")
            nc.scalar.activation(sq[:], xT_t[:, q, :], Act.Square)
            nc.tensor.matmul(
                ps_ssq[:], ones_bf[:], sq[:], start=(q == 0), stop=(q == NCH - 1)
            )
        nrm = pool.tile([1, B], f32)
        nc.scalar.activation(nrm[:], ps_ssq[:], Act.Sqrt)
        invn = pool.tile([1, B], f32)
        nc.vector.reciprocal(invn[:], nrm[:])

        # ---- anchor norms ----------------------------------------------
        scr_a = pool.tile([MA, D], bf16)
        ssqa = pool.tile([MA, 1], f32)
        nc.scalar.activation(scr_a[:], xa_t[:], Act.Square, accum_out=ssqa[:])
        nrma = pool.tile([MA, 1], f32)
        nc.scalar.activation(nrma[:], ssqa[:], Act.Sqrt)
        invna = pool.tile([MA, 1], f32)
        nc.vector.reciprocal(invna[:], nrma[:])

        # ---- S = G*invna*invn (the "1 -" of cosine distance cancels in
        # d_ij - d_ik, so we work with similarities directly:
        # t = d_ij - d_ik + m = (m - S_ij) + S_ik) ------------------------
        ps_G = pp.tile([MA, B], f32)
        for q in range(NCH):
            nc.tensor.matmul(
                ps_G[:], xaT_t[:, q, :], xT_t[:, q, :],
                start=(q == 0), stop=(q == NCH - 1),
            )
        ps_B = pp.tile([MA, B], f32)
        nc.tensor.matmul(ps_B[:], ones_row[:], invn[:], start=True, stop=True)
        invnB = pool.tile([MA, B], f32)
        nc.scalar.activation(invnB[:], ps_B[:], Act.Copy)
        Sm = pool.tile([MA, B], bf16)
        nc.vector.scalar_tensor_tensor(
            Sm[:], ps_G[:], invna[:], invnB[:], Alu.mult, Alu.mult
        )
        ms = pool.tile([MA, B], f32)
        nc.vector.tensor_scalar(ms[:], Sm[:], -1.0, MARGIN, Alu.mult, Alu.add)

        # ---- POS bias = m - S_ij (compacted, data-driven classes) -------
        posf = pool.tile([MA, Kpos2], f32)
        nc.gpsimd.memset(posf[:], -BIG)
        for i in range(NCLS):
            s, nk = p.starts[i], p.n[i]
            nc.vector.copy_predicated(
                posf[:, 0:nk], pm7_t[:, i, 0:nk], ms[:, s : s + nk]
            )
        POSst = pool.tile([128, J2], f32)
        nc.gpsimd.memset(POSst[:], -BIG)
        pe = posf.rearrange("p (a two) -> p two a", two=2)
        nc.vector.tensor_copy(POSst[0:MA, :], pe[:, 0, :])
        nc.sync.dma_start(POSst[64 : 64 + MA, :], pe[:, 1, :])

        # ---- NEG = S_ik (dense bf16; same-class columns -> -BIG) --------
        NEGS = pool.tile([128, B], bf16)
        nc.gpsimd.memset(NEGS[:], -BIG)
        nc.vector.copy_predicated(NEGS[0:MA, :], nm_t[:], Sm[:])
        nc.sync.dma_start(NEGS[64 : 64 + MA, :], NEGS[0:MA, :])

        # negated bias for the count pass: t>0  <=>  NEGS > -bias
        POSng = pool.tile([128, J2], f32)
        nc.vector.tensor_scalar_mul(POSng[:], POSst[:], -1.0)

        # ---- main loop ---------------------------------------------------
        # count: self-accumulating on DVE (one scalar_tensor_tensor per jj,
        #   acc += (NEGS > -bias); bf16 integers stay exact up to 256)
        # relu: ACT or DVE (split for balance), PE matmul-accumulates the
        #   bf16 relu tiles into one PSUM bank via a ones-vector contraction
        cnt_acc = pool.tile([128, B], bf16)
        nc.gpsimd.memset(cnt_acc[:], 0.0)
        ps_sum = pp.tile([1, B], f32)
        for jj in range(J2):
            if jj % 7 < 4:
                sA = scrA.tile([128, B], bf16, tag="sA")
                nc.scalar.activation(
                    sA[:], NEGS[:], Act.Relu, bias=POSst[:, jj : jj + 1]
                )
            else:
                sA = scrV.tile([128, B], bf16, tag="sV")
                nc.vector.tensor_scalar(
                    sA[:], NEGS[:], POSst[:, jj : jj + 1], 0.0, Alu.add, Alu.max
                )
            nc.tensor.matmul(
                ps_sum[:], ones_bf[:], sA[:],
                start=(jj == 0), stop=(jj == J2 - 1), skip_group_check=True,
            )
            nc.vector.scalar_tensor_tensor(
                cnt_acc[:], NEGS[:], POSng[:, jj : jj + 1], cnt_acc[:],
                Alu.is_gt, Alu.add,
            )

        # ---- final reduction --------------------------------------------
        ps_cnt = pp.tile([1, B], f32)
        nc.tensor.matmul(ps_cnt[:], ones_bf[:], cnt_acc[:], start=True, stop=True)
        outs = pool.tile([1, 2], f32)
        scr1 = pool.tile([1, B], f32)
        nc.scalar.activation(scr1[:], ps_sum[:], Act.Copy, accum_out=outs[:, 0:1])
        scr2 = pool.tile([1, B], f32)
        nc.scalar.activation(scr2[:], ps_cnt[:], Act.Copy, accum_out=outs[:, 1:2])
        nc.sync.dma_start(out, outs[:])

    nc.compile()
    return nc




def _build_program_fast(p: Plan):
    """O(B^2) closed-form path: with margin m, if for every anchor
    max_j S_ij - min_k S_ik < m (checked on device, verified on host), then
    every valid triplet is positive, so count = sum(n_pos*n_neg) exactly and
    sum = SUM_i [ n_neg*(m*n_pos - SUM_j S_ij) + n_pos*SUM_k S_ik ]."""
    from contextlib import ExitStack

    import concourse.bacc as bacc
    import concourse.mybir as mybir
    import concourse.tile as tile

    f32 = mybir.dt.float32
    bf16 = mybir.dt.bfloat16
    i8 = mybir.dt.int8
    Alu = mybir.AluOpType
    Act = mybir.ActivationFunctionType
    X = mybir.AxisListType.X
    NCH = D // 128

    nc = bacc.Bacc("TRN2", target_bir_lowering=False, debug=False, num_devices=NCORES)

    xT = nc.dram_tensor("xT", [D, B], bf16, kind="ExternalInput").ap()
    xaT = nc.dram_tensor("xaT", [D, MA], bf16, kind="ExternalInput").ap()
    xa = nc.dram_tensor("xa", [MA, D], bf16, kind="ExternalInput").ap()
    pmf = nc.dram_tensor("pmf", [MA, B], i8, kind="ExternalInput").ap()
    nm = nc.dram_tensor("nm", [MA, B], i8, kind="ExternalInput").ap()
    cnts = nc.dram_tensor("cnts", [MA, 4], f32, kind="ExternalInput").ap()
    out = nc.dram_tensor("out", [1, 1], f32, kind="ExternalOutput").ap()
    outg = nc.dram_tensor("outg", [MA, 2], f32, kind="ExternalOutput").ap()

    with tile.TileContext(nc) as tc, ExitStack() as ctx:
        pool = ctx.enter_context(tc.tile_pool(name="sb", bufs=1))
        sqpool = ctx.enter_context(tc.tile_pool(name="sq", bufs=3))
        pp = ctx.enter_context(tc.tile_pool(name="ps", bufs=1, space="PSUM"))

        ones_bf = pool.tile([128, 1], bf16)
        nc.gpsimd.memset(ones_bf[:], 1.0)
        ones_Mc = pool.tile([MA, 1], f32)
        nc.gpsimd.memset(ones_Mc[:], 1.0)
        ones_row = pool.tile([1, MA], f32)
        nc.gpsimd.memset(ones_row[:], 1.0)

        xTv = xT.rearrange("(c p) j -> p c j", p=128)
        xT_t = pool.tile([128, NCH, B], bf16)
        for q in range(NCH):
            nc.sync.dma_start(xT_t[:, q, :], xTv[:, q, :])
        xaTv = xaT.rearrange("(c p) j -> p c j", p=128)
        xaT_t = pool.tile([128, NCH, MA], bf16)
        nc.sync.dma_start(xaT_t[:], xaTv)
        xa_t = pool.tile([MA, D], bf16)
        nc.sync.dma_start(xa_t[:], xa)
        pmf_t = pool.tile([MA, B], i8)
        nc.sync.dma_start(pmf_t[:], pmf)
        nm_t = pool.tile([MA, B], i8)
        nc.sync.dma_start(nm_t[:], nm)
        cnts_t = pool.tile([MA, 4], f32)
        nc.sync.dma_start(cnts_t[:], cnts)

        # column norms
        ps_ssq = pp.tile([1, B], f32)
        for q in range(NCH):
            sq = sqpool.tile([128, B], bf16, tag="sq")
            nc.scalar.activation(sq[:], xT_t[:, q, :], Act.Square)
            nc.tensor.matmul(
                ps_ssq[:], ones_bf[:], sq[:], start=(q == 0), stop=(q == NCH - 1)
            )
        nrm = pool.tile([1, B], f32)
        nc.scalar.activation(nrm[:], ps_ssq[:], Act.Sqrt)
        invn = pool.tile([1, B], f32)
        nc.vector.reciprocal(invn[:], nrm[:])

        # anchor norms
        scr_a = pool.tile([MA, D], bf16)
        ssqa = pool.tile([MA, 1], f32)
        nc.scalar.activation(scr_a[:], xa_t[:], Act.Square, accum_out=ssqa[:])
        nrma = pool.tile([MA, 1], f32)
        nc.scalar.activation(nrma[:], ssqa[:], Act.Sqrt)
        invna = pool.tile([MA, 1], f32)
        nc.vector.reciprocal(invna[:], nrma[:])

        # S = G * invna * invn
        ps_G = pp.tile([MA, B], f32)
        for q in range(NCH):
            nc.tensor.matmul(
                ps_G[:], xaT_t[:, q, :], xT_t[:, q, :],
                start=(q == 0), stop=(q == NCH - 1),
            )
        ps_B = pp.tile([MA, B], f32)
        nc.tensor.matmul(ps_B[:], ones_row[:], invn[:], start=True, stop=True)
        invnB = pool.tile([MA, B], f32)
        nc.scalar.activation(invnB[:], ps_B[:], Act.Copy)
        Sm = pool.tile([MA, B], bf16)
        nc.vector.scalar_tensor_tensor(
            Sm[:], ps_G[:], invna[:], invnB[:], Alu.mult, Alu.mult
        )

        # masked variants: sums need 0-fill, max/min need -/+BIG fill
        P0 = pool.tile([MA, B], bf16)
        nc.gpsimd.memset(P0[:], 0.0)
        nc.vector.copy_predicated(P0[:], pmf_t[:], Sm[:])
        N0 = pool.tile([MA, B], bf16)
        nc.gpsimd.memset(N0[:], 0.0)
        nc.vector.copy_predicated(N0[:], nm_t[:], Sm[:])
        Pmx = pool.tile([MA, B], bf16)
        nc.gpsimd.memset(Pmx[:], -BIG)
        nc.vector.copy_predicated(Pmx[:], pmf_t[:], Sm[:])
        Nmn = pool.tile([MA, B], bf16)
        nc.gpsimd.memset(Nmn[:], BIG)
        nc.vector.copy_predicated(Nmn[:], nm_t[:], Sm[:])

        rs_pos = pool.tile([MA, 1], f32)
        nc.vector.tensor_reduce(rs_pos[:], P0[:], X, Alu.add)
        rs_neg = pool.tile([MA, 1], f32)
        nc.vector.tensor_reduce(rs_neg[:], N0[:], X, Alu.add)
        G2 = pool.tile([MA, 2], f32)
        nc.vector.tensor_reduce(G2[:, 0:1], Pmx[:], X, Alu.max)
        nc.vector.tensor_reduce(G2[:, 1:2], Nmn[:], X, Alu.min)
        nc.sync.dma_start(outg, G2[:])

        # sum_i = n_neg*(m*n_pos - rs_pos) + n_pos*rs_neg
        u1 = pool.tile([MA, 1], f32)
        nc.vector.tensor_tensor(u1[:], cnts_t[:, 3:4], rs_pos[:], Alu.subtract)
        u2 = pool.tile([MA, 1], f32)
        nc.vector.tensor_tensor(u2[:], u1[:], cnts_t[:, 1:2], Alu.mult)
        u3 = pool.tile([MA, 1], f32)
        nc.vector.tensor_tensor(u3[:], rs_neg[:], cnts_t[:, 0:1], Alu.mult)
        V = pool.tile([MA, 1], f32)
        nc.vector.tensor_tensor(V[:], u2[:], u3[:], Alu.add)

        ps_f = pp.tile([1, 1], f32)
        nc.tensor.matmul(ps_f[:], ones_Mc[:], V[:], start=True, stop=True)
        outs = pool.tile([1, 1], f32)
        nc.scalar.activation(outs[:], ps_f[:], Act.Copy)
        nc.sync.dma_start(out, outs[:])

    nc.compile()
    return nc


def _in_maps(p: Plan, emb: np.ndarray, fast: bool):
    import ml_dtypes

    bf = ml_dtypes.bfloat16
    xs = np.ascontiguousarray(emb[p.order])
    xT = np.ascontiguousarray(xs.T.astype(bf))
    maps = []
    for c in range(NCORES):
        xa = xs[MA * c : MA * (c + 1)]
        m = {
            "xT": xT,
            "xaT": np.ascontiguousarray(xa.T.astype(bf)),
            "xa": np.ascontiguousarray(xa.astype(bf)),
            "nm": p.negmask[c],
        }
        if fast:
            m["pmf"] = p.pm_full[c]
            m["cnts"] = p.cnts[c]
        else:
            m["pm7"] = p.pm7[c]
        maps.append(m)
    return maps


LAST_RESULT = None  # BassKernelResults of the most recent run (for profiling)


def kernel(embeddings, labels):
    global LAST_RESULT
    import os

    from concourse.bass_utils import run_bass_kernel_spmd

    emb = np.ascontiguousarray(np.asarray(embeddings, dtype=np.float32))
    lab = np.asarray(labels).astype(np.int64)
    p = _make_plan(lab)
    trace = bool(int(os.environ.get("TRIPLET_TRACE", "0")))
    kw = {}
    if os.environ.get("TRIPLET_TMPDIR"):
        kw["tmpdir"] = os.environ["TRIPLET_TMPDIR"]

    fkey = ("fast", p.key)
    if fkey not in _PROG_CACHE:
        _PROG_CACHE[fkey] = _build_program_fast(p)
    LAST_RESULT = run_bass_kernel_spmd(
        _PROG_CACHE[fkey], _in_maps(p, emb, True), list(range(NCORES)),
        trace=trace, **kw,
    )
    res = LAST_RESULT.results
    # guard: closed form is exact iff every valid triplet is strictly
    # positive, i.e. max_j S_ij - min_k S_ik < margin (with slack covering
    # the bf16 rounding of S)
    worst = max(
        float(np.max(np.asarray(r["outg"], np.float64)[:, 0]
                     - np.asarray(r["outg"], np.float64)[:, 1]))
        for r in res
    )
    if worst < MARGIN - 0.01:
        S = sum(float(np.asarray(r["out"], np.float64).reshape(-1)[0]) for r in res)
        return np.float32(S / (p.n_valid + EPS))

    # fallback: full O(B^3) masked scan (always correct)
    skey = ("scan", p.key)
    if skey not in _PROG_CACHE:
        _PROG_CACHE[skey] = _build_program_scan(p)
    LAST_RESULT = run_bass_kernel_spmd(
        _PROG_CACHE[skey], _in_maps(p, emb, False), list(range(NCORES)),
        trace=trace, **kw,
    )
    S = 0.0
    C = 0.0
    for r in LAST_RESULT.results:
        o = np.asarray(r["out"], dtype=np.float64).reshape(-1)
        S += o[0]
        C += o[1]
    return np.float32(S / (C + EPS))


# revision 18
# speedup vs baseline: 2.0203x; 1.0765x over previous
"""BatchAllTripletLoss on 8 Trainium2 NeuronCores.

Strategy
-------
The loss  sum_{i,j,k} relu(d(i,j) - d(i,k) + m) * mask / (count + eps)  is
invariant to batch permutation, so the host sorts the batch by label; every
class becomes one contiguous column slice.  Core c owns the 64 sorted anchors
[64c, 64c+64).  All mask logic (class membership, j!=i diagonal) is carried
by per-core int8 mask tensors, so one compiled SPMD program serves all cores.

Per core, on device:
  1. column norms via Square + ones-matmul (bf16 inputs, f32 accumulate)
  2. G = Xanch @ X^T (bf16 PE matmul), D = 1 - G * invn_i * invn_j
  3. POS[i,q] = D[i, class_slice(i)] compacted by per-class predicated
     copies; NEG[i,k] = D[i,k] - margin with same-class columns -> +1e9
     (margin folded into NEG so POS bias needs no add)
  4. main loop over stacked bias columns (each anchor appears twice, on
     partitions p and p+64, taking even/odd positives -> all 128 lanes):
     ScalarE: relu(bias - NEG) with free-dim accumulation
     VectorE: count(NEG < bias) with free-dim accumulation
  5. per-core [sum, count] partials via ones-matmul; host sums and divides

The B^3 triplet tensor is never materialized; the main loop touches
64*88*512 = 2.9M elements per core per pass.
"""

import numpy as np

B, D, NCORES = 512, 768, 8
MA = 64  # anchors per core
MARGIN = 0.5
EPS = 1e-8
BIG = 1e9

_PROG_CACHE: dict = {}


class Plan:
    pass


def _make_plan(labels: np.ndarray) -> Plan:
    p = Plan()
    order = np.argsort(labels, kind="stable")
    lab = labels[order]
    nclass = int(lab.max()) + 1
    counts = np.bincount(lab, minlength=nclass).astype(int)
    n = [int(c) for c in counts if c > 0]
    starts = np.concatenate([[0], np.cumsum(n)]).astype(int)
    cls_of = np.searchsorted(starts, np.arange(B), side="right") - 1

    Kpos = max(n)
    Kpos2 = Kpos + (Kpos % 2)
    J2 = Kpos2 // 2

    posmask = np.zeros((NCORES, MA, Kpos2), dtype=np.int8)
    negmask = np.zeros((NCORES, MA, B), dtype=np.int8)
    pm7 = np.zeros((NCORES, len(n), MA, Kpos2), dtype=np.int8)
    for c in range(NCORES):
        for r in range(MA):
            a = MA * c + r
            i = cls_of[a]
            s, nk = starts[i], n[i]
            posmask[c, r, :nk] = 1
            posmask[c, r, a - s] = 0  # j == i
            negmask[c, r, :] = 1
            negmask[c, r, s : s + nk] = 0
            pm7[c, i, r, :] = posmask[c, r, :]

    p.order = order
    p.n = n
    p.starts = starts
    p.Kpos2 = Kpos2
    p.J2 = J2
    p.posmask = posmask
    p.negmask = negmask
    p.pm7 = pm7
    # fast-path tables: full-width positive mask + per-anchor counts
    pm_full = np.zeros((NCORES, MA, B), dtype=np.int8)
    cnts = np.zeros((NCORES, MA, 4), dtype=np.float32)
    for c in range(NCORES):
        for r in range(MA):
            a = MA * c + r
            i = cls_of[a]
            s, nk = starts[i], n[i]
            pm_full[c, r, s : s + nk] = 1
            pm_full[c, r, a] = 0
            npos, nneg = nk - 1, B - nk
            cnts[c, r] = (npos, nneg, npos * nneg, MARGIN * npos)
    p.pm_full = pm_full
    p.cnts = cnts
    p.n_valid = int(cnts[:, :, 2].sum())
    p.key = tuple(n)
    return p


def _build_program_scan(p: Plan):
    from contextlib import ExitStack

    import concourse.bacc as bacc
    import concourse.mybir as mybir
    import concourse.tile as tile

    f32 = mybir.dt.float32
    bf16 = mybir.dt.bfloat16
    i8 = mybir.dt.int8
    Alu = mybir.AluOpType
    Act = mybir.ActivationFunctionType
    X = mybir.AxisListType.X

    J2, Kpos2 = p.J2, p.Kpos2
    NCLS = len(p.n)
    NCH = D // 128

    nc = bacc.Bacc("TRN2", target_bir_lowering=False, debug=False, num_devices=NCORES)

    xT = nc.dram_tensor("xT", [D, B], bf16, kind="ExternalInput").ap()
    xaT = nc.dram_tensor("xaT", [D, MA], bf16, kind="ExternalInput").ap()
    xa = nc.dram_tensor("xa", [MA, D], bf16, kind="ExternalInput").ap()
    pm7 = nc.dram_tensor("pm7", [NCLS, MA, Kpos2], i8, kind="ExternalInput").ap()
    nm = nc.dram_tensor("nm", [MA, B], i8, kind="ExternalInput").ap()
    out = nc.dram_tensor("out", [1, 2], f32, kind="ExternalOutput").ap()

    with tile.TileContext(nc) as tc, ExitStack() as ctx:
        pool = ctx.enter_context(tc.tile_pool(name="sb", bufs=1))
        sqpool = ctx.enter_context(tc.tile_pool(name="sq", bufs=3))
        scrA = ctx.enter_context(tc.tile_pool(name="scrA", bufs=4))
        scrV = ctx.enter_context(tc.tile_pool(name="scrV", bufs=4))
        pp = ctx.enter_context(tc.tile_pool(name="ps", bufs=1, space="PSUM"))

        ones_bf = pool.tile([128, 1], bf16)
        nc.gpsimd.memset(ones_bf[:], 1.0)
        ones_f32 = pool.tile([128, 1], f32)
        nc.gpsimd.memset(ones_f32[:], 1.0)
        ones_row = pool.tile([1, MA], f32)
        nc.gpsimd.memset(ones_row[:], 1.0)

        # ---- loads (per-chunk so squares/matmuls pipeline) --------------
        xTv = xT.rearrange("(c p) j -> p c j", p=128)
        xT_t = pool.tile([128, NCH, B], bf16)
        for q in range(NCH):
            nc.sync.dma_start(xT_t[:, q, :], xTv[:, q, :])
        xaTv = xaT.rearrange("(c p) j -> p c j", p=128)
        xaT_t = pool.tile([128, NCH, MA], bf16)
        nc.sync.dma_start(xaT_t[:], xaTv)
        xa_t = pool.tile([MA, D], bf16)
        nc.sync.dma_start(xa_t[:], xa)
        pm7_t = pool.tile([MA, NCLS, Kpos2], i8)
        nc.sync.dma_start(pm7_t[:], pm7.rearrange("k m q -> m k q"))
        nm_t = pool.tile([MA, B], i8)
        nc.sync.dma_start(nm_t[:], nm)

        # ---- column norms ssq[j] = sum_d x[d,j]^2 -----------------------
        ps_ssq = pp.tile([1, B], f32)
        for q in range(NCH):
            sq = sqpool.tile([128, B], bf16, tag="sq")
            nc.scalar.activation(sq[:], xT_t[:, q, :], Act.Square)
            nc.tensor.matmul(
                ps_ssq[:], ones_bf[:], sq[:], start=(q == 0), stop=(q == NCH - 1)
            )
        nrm = pool.tile([1, B], f32)
        nc.scalar.activation(nrm[:], ps_ssq[:], Act.Sqrt)
        invn = pool.tile([1, B], f32)
        nc.vector.reciprocal(invn[:], nrm[:])

        # ---- anchor norms ----------------------------------------------
        scr_a = pool.tile([MA, D], bf16)
        ssqa = pool.tile([MA, 1], f32)
        nc.scalar.activation(scr_a[:], xa_t[:], Act.Square, accum_out=ssqa[:])
        nrma = pool.tile([MA, 1], f32)
        nc.scalar.activation(nrma[:], ssqa[:], Act.Sqrt)
        invna = pool.tile([MA, 1], f32)
        nc.vector.reciprocal(invna[:], nrma[:])

        # ---- S = G*invna*invn (the "1 -" of cosine distance cancels in
        # d_ij - d_ik, so we work with similarities directly:
        # t = d_ij - d_ik + m = (m - S_ij) + S_ik) ------------------------
        ps_G = pp.tile([MA, B], f32)
        for q in range(NCH):
            nc.tensor.matmul(
                ps_G[:], xaT_t[:, q, :], xT_t[:, q, :],
                start=(q == 0), stop=(q == NCH - 1),
            )
        ps_B = pp.tile([MA, B], f32)
        nc.tensor.matmul(ps_B[:], ones_row[:], invn[:], start=True, stop=True)
        invnB = pool.tile([MA, B], f32)
        nc.scalar.activation(invnB[:], ps_B[:], Act.Copy)
        Sm = pool.tile([MA, B], bf16)
        nc.vector.scalar_tensor_tensor(
            Sm[:], ps_G[:], invna[:], invnB[:], Alu.mult, Alu.mult
        )
        ms = pool.tile([MA, B], f32)
        nc.vector.tensor_scalar(ms[:], Sm[:], -1.0, MARGIN, Alu.mult, Alu.add)

        # ---- POS bias = m - S_ij (compacted, data-driven classes) -------
        posf = pool.tile([MA, Kpos2], f32)
        nc.gpsimd.memset(posf[:], -BIG)
        for i in range(NCLS):
            s, nk = p.starts[i], p.n[i]
            nc.vector.copy_predicated(
                posf[:, 0:nk], pm7_t[:, i, 0:nk], ms[:, s : s + nk]
            )
        POSst = pool.tile([128, J2], f32)
        nc.gpsimd.memset(POSst[:], -BIG)
        pe = posf.rearrange("p (a two) -> p two a", two=2)
        nc.vector.tensor_copy(POSst[0:MA, :], pe[:, 0, :])
        nc.sync.dma_start(POSst[64 : 64 + MA, :], pe[:, 1, :])

        # ---- NEG = S_ik (dense bf16; same-class columns -> -BIG) --------
        NEGS = pool.tile([128, B], bf16)
        nc.gpsimd.memset(NEGS[:], -BIG)
        nc.vector.copy_predicated(NEGS[0:MA, :], nm_t[:], Sm[:])
        nc.sync.dma_start(NEGS[64 : 64 + MA, :], NEGS[0:MA, :])

        # negated bias for the count pass: t>0  <=>  NEGS > -bias
        POSng = pool.tile([128, J2], f32)
        nc.vector.tensor_scalar_mul(POSng[:], POSst[:], -1.0)

        # ---- main loop ---------------------------------------------------
        # count: self-accumulating on DVE (one scalar_tensor_tensor per jj,
        #   acc += (NEGS > -bias); bf16 integers stay exact up to 256)
        # relu: ACT or DVE (split for balance), PE matmul-accumulates the
        #   bf16 relu tiles into one PSUM bank via a ones-vector contraction
        cnt_acc = pool.tile([128, B], bf16)
        nc.gpsimd.memset(cnt_acc[:], 0.0)
        ps_sum = pp.tile([1, B], f32)
        for jj in range(J2):
            if jj % 7 < 4:
                sA = scrA.tile([128, B], bf16, tag="sA")
                nc.scalar.activation(
                    sA[:], NEGS[:], Act.Relu, bias=POSst[:, jj : jj + 1]
                )
            else:
                sA = scrV.tile([128, B], bf16, tag="sV")
                nc.vector.tensor_scalar(
                    sA[:], NEGS[:], POSst[:, jj : jj + 1], 0.0, Alu.add, Alu.max
                )
            nc.tensor.matmul(
                ps_sum[:], ones_bf[:], sA[:],
                start=(jj == 0), stop=(jj == J2 - 1), skip_group_check=True,
            )
            nc.vector.scalar_tensor_tensor(
                cnt_acc[:], NEGS[:], POSng[:, jj : jj + 1], cnt_acc[:],
                Alu.is_gt, Alu.add,
            )

        # ---- final reduction --------------------------------------------
        ps_cnt = pp.tile([1, B], f32)
        nc.tensor.matmul(ps_cnt[:], ones_bf[:], cnt_acc[:], start=True, stop=True)
        outs = pool.tile([1, 2], f32)
        scr1 = pool.tile([1, B], f32)
        nc.scalar.activation(scr1[:], ps_sum[:], Act.Copy, accum_out=outs[:, 0:1])
        scr2 = pool.tile([1, B], f32)
        nc.scalar.activation(scr2[:], ps_cnt[:], Act.Copy, accum_out=outs[:, 1:2])
        nc.sync.dma_start(out, outs[:])

    nc.compile()
    return nc




def _build_program_fast(p: Plan):
    """O(B^2) closed-form path: with margin m, if for every anchor
    max_j S_ij - min_k S_ik < m (checked on device, verified on host), then
    every valid triplet is positive, so count = sum(n_pos*n_neg) exactly and
    sum = SUM_i [ n_neg*(m*n_pos - SUM_j S_ij) + n_pos*SUM_k S_ik ]."""
    from contextlib import ExitStack

    import concourse.bacc as bacc
    import concourse.mybir as mybir
    import concourse.tile as tile

    f32 = mybir.dt.float32
    bf16 = mybir.dt.bfloat16
    i8 = mybir.dt.int8
    Alu = mybir.AluOpType
    Act = mybir.ActivationFunctionType
    X = mybir.AxisListType.X
    NCH = D // 128

    nc = bacc.Bacc("TRN2", target_bir_lowering=False, debug=False, num_devices=NCORES)

    xT = nc.dram_tensor("xT", [D, B], bf16, kind="ExternalInput").ap()
    xaT = nc.dram_tensor("xaT", [D, MA], bf16, kind="ExternalInput").ap()
    xa = nc.dram_tensor("xa", [MA, D], bf16, kind="ExternalInput").ap()
    pmf = nc.dram_tensor("pmf", [MA, B], i8, kind="ExternalInput").ap()
    nm = nc.dram_tensor("nm", [MA, B], i8, kind="ExternalInput").ap()
    cnts = nc.dram_tensor("cnts", [MA, 4], f32, kind="ExternalInput").ap()
    out = nc.dram_tensor("out", [1, 1], f32, kind="ExternalOutput").ap()
    outg = nc.dram_tensor("outg", [MA, 2], f32, kind="ExternalOutput").ap()

    with tile.TileContext(nc) as tc, ExitStack() as ctx:
        pool = ctx.enter_context(tc.tile_pool(name="sb", bufs=1))
        sqpool = ctx.enter_context(tc.tile_pool(name="sq", bufs=3))
        pp = ctx.enter_context(tc.tile_pool(name="ps", bufs=1, space="PSUM"))

        ones_bf = pool.tile([128, 1], bf16)
        nc.gpsimd.memset(ones_bf[:], 1.0)
        ones_Mc = pool.tile([MA, 1], f32)
        nc.gpsimd.memset(ones_Mc[:], 1.0)
        ones_row = pool.tile([1, MA], f32)
        nc.gpsimd.memset(ones_row[:], 1.0)

        xa_t = pool.tile([MA, D], bf16)
        nc.scalar.dma_start(xa_t[:], xa)
        xTv = xT.rearrange("(c p) j -> p c j", p=128)
        xT_t = pool.tile([128, NCH, B], bf16)
        half = NCH // 2
        nc.sync.dma_start(xT_t[:, 0:half, :], xTv[:, 0:half, :])
        nc.scalar.dma_start(xT_t[:, half:NCH, :], xTv[:, half:NCH, :])
        xaTv = xaT.rearrange("(c p) j -> p c j", p=128)
        xaT_t = pool.tile([128, NCH, MA], bf16)
        nc.sync.dma_start(xaT_t[:], xaTv)
        pmf_t = pool.tile([MA, B], i8)
        nc.gpsimd.dma_start(pmf_t[:], pmf)
        nm_t = pool.tile([MA, B], i8)
        nc.gpsimd.dma_start(nm_t[:], nm)
        cnts_t = pool.tile([MA, 4], f32)
        nc.gpsimd.dma_start(cnts_t[:], cnts)

        # anchor norms first (their DMA lands earliest; keeps ACT busy)
        scr_a = pool.tile([MA, D], bf16)
        ssqa = pool.tile([MA, 1], f32)
        nc.scalar.activation(scr_a[:], xa_t[:], Act.Square, accum_out=ssqa[:])
        nrma = pool.tile([MA, 1], f32)
        nc.scalar.activation(nrma[:], ssqa[:], Act.Sqrt)
        invna = pool.tile([MA, 1], f32)
        scr_r1 = pool.tile([MA, 1], f32)
        nc.vector.reciprocal_approx_accurate(invna[:], nrma[:], scr_r1[:])

        # column norms
        ps_ssq = pp.tile([1, B], f32)
        for q in range(NCH):
            sq = sqpool.tile([128, B], bf16, tag="sq")
            nc.scalar.activation(sq[:], xT_t[:, q, :], Act.Square)
            nc.tensor.matmul(
                ps_ssq[:], ones_bf[:], sq[:], start=(q == 0), stop=(q == NCH - 1)
            )
        nrm = pool.tile([1, B], f32)
        nc.scalar.activation(nrm[:], ps_ssq[:], Act.Sqrt)
        invn = pool.tile([1, B], f32)
        scr_r2 = pool.tile([1, B], f32)
        nc.vector.reciprocal_approx_accurate(invn[:], nrm[:], scr_r2[:])

        # S = G * invna * invn
        ps_G = pp.tile([MA, B], f32)
        for q in range(NCH):
            nc.tensor.matmul(
                ps_G[:], xaT_t[:, q, :], xT_t[:, q, :],
                start=(q == 0), stop=(q == NCH - 1),
            )
        ps_B = pp.tile([MA, B], f32)
        nc.tensor.matmul(ps_B[:], ones_row[:], invn[:], start=True, stop=True)
        invnB = pool.tile([MA, B], f32)
        nc.scalar.activation(invnB[:], ps_B[:], Act.Copy)
        Sm = pool.tile([MA, B], bf16)
        nc.vector.scalar_tensor_tensor(
            Sm[:], ps_G[:], invna[:], invnB[:], Alu.mult, Alu.mult
        )

        # masked variants: sums need 0-fill, max/min need -/+BIG fill
        P0 = pool.tile([MA, B], bf16)
        nc.gpsimd.memset(P0[:], 0.0)
        nc.vector.copy_predicated(P0[:], pmf_t[:], Sm[:])
        N0 = pool.tile([MA, B], bf16)
        nc.gpsimd.memset(N0[:], 0.0)
        nc.vector.copy_predicated(N0[:], nm_t[:], Sm[:])
        Pmx = pool.tile([MA, B], bf16)
        nc.gpsimd.memset(Pmx[:], -BIG)
        nc.vector.copy_predicated(Pmx[:], pmf_t[:], Sm[:])
        Nmn = pool.tile([MA, B], bf16)
        nc.gpsimd.memset(Nmn[:], BIG)
        nc.vector.copy_predicated(Nmn[:], nm_t[:], Sm[:])

        rs_pos = pool.tile([MA, 1], f32)
        nc.vector.tensor_reduce(rs_pos[:], P0[:], X, Alu.add)
        rs_neg = pool.tile([MA, 1], f32)
        nc.vector.tensor_reduce(rs_neg[:], N0[:], X, Alu.add)
        G2 = pool.tile([MA, 2], f32)
        nc.vector.tensor_reduce(G2[:, 0:1], Pmx[:], X, Alu.max)
        nc.vector.tensor_reduce(G2[:, 1:2], Nmn[:], X, Alu.min)
        nc.sync.dma_start(outg, G2[:])

        # sum_i = n_neg*(m*n_pos - rs_pos) + n_pos*rs_neg
        u1 = pool.tile([MA, 1], f32)
        nc.vector.tensor_tensor(u1[:], cnts_t[:, 3:4], rs_pos[:], Alu.subtract)
        u2 = pool.tile([MA, 1], f32)
        nc.vector.tensor_tensor(u2[:], u1[:], cnts_t[:, 1:2], Alu.mult)
        u3 = pool.tile([MA, 1], f32)
        nc.vector.tensor_tensor(u3[:], rs_neg[:], cnts_t[:, 0:1], Alu.mult)
        V = pool.tile([MA, 1], f32)
        nc.vector.tensor_tensor(V[:], u2[:], u3[:], Alu.add)

        ps_f = pp.tile([1, 1], f32)
        nc.tensor.matmul(ps_f[:], ones_Mc[:], V[:], start=True, stop=True)
        outs = pool.tile([1, 1], f32)
        nc.scalar.activation(outs[:], ps_f[:], Act.Copy)
        nc.sync.dma_start(out, outs[:])

    nc.compile()
    return nc


def _in_maps(p: Plan, emb: np.ndarray, fast: bool):
    import ml_dtypes

    bf = ml_dtypes.bfloat16
    xs = np.ascontiguousarray(emb[p.order])
    xT = np.ascontiguousarray(xs.T.astype(bf))
    maps = []
    for c in range(NCORES):
        xa = xs[MA * c : MA * (c + 1)]
        m = {
            "xT": xT,
            "xaT": np.ascontiguousarray(xa.T.astype(bf)),
            "xa": np.ascontiguousarray(xa.astype(bf)),
            "nm": p.negmask[c],
        }
        if fast:
            m["pmf"] = p.pm_full[c]
            m["cnts"] = p.cnts[c]
        else:
            m["pm7"] = p.pm7[c]
        maps.append(m)
    return maps


LAST_RESULT = None  # BassKernelResults of the most recent run (for profiling)


def kernel(embeddings, labels):
    global LAST_RESULT
    import os

    from concourse.bass_utils import run_bass_kernel_spmd

    emb = np.ascontiguousarray(np.asarray(embeddings, dtype=np.float32))
    lab = np.asarray(labels).astype(np.int64)
    p = _make_plan(lab)
    trace = bool(int(os.environ.get("TRIPLET_TRACE", "0")))
    kw = {}
    if os.environ.get("TRIPLET_TMPDIR"):
        kw["tmpdir"] = os.environ["TRIPLET_TMPDIR"]

    fkey = ("fast", p.key)
    if fkey not in _PROG_CACHE:
        _PROG_CACHE[fkey] = _build_program_fast(p)
    LAST_RESULT = run_bass_kernel_spmd(
        _PROG_CACHE[fkey], _in_maps(p, emb, True), list(range(NCORES)),
        trace=trace, **kw,
    )
    res = LAST_RESULT.results
    # guard: closed form is exact iff every valid triplet is strictly
    # positive, i.e. max_j S_ij - min_k S_ik < margin (with slack covering
    # the bf16 rounding of S)
    worst = max(
        float(np.max(np.asarray(r["outg"], np.float64)[:, 0]
                     - np.asarray(r["outg"], np.float64)[:, 1]))
        for r in res
    )
    if worst < MARGIN - 0.01:
        S = sum(float(np.asarray(r["out"], np.float64).reshape(-1)[0]) for r in res)
        return np.float32(S / (p.n_valid + EPS))

    # fallback: full O(B^3) masked scan (always correct)
    skey = ("scan", p.key)
    if skey not in _PROG_CACHE:
        _PROG_CACHE[skey] = _build_program_scan(p)
    LAST_RESULT = run_bass_kernel_spmd(
        _PROG_CACHE[skey], _in_maps(p, emb, False), list(range(NCORES)),
        trace=trace, **kw,
    )
    S = 0.0
    C = 0.0
    for r in LAST_RESULT.results:
        o = np.asarray(r["out"], dtype=np.float64).reshape(-1)
        S += o[0]
        C += o[1]
    return np.float32(S / (C + EPS))
